# revision 11
# baseline (speedup 1.0000x reference)
"""Trainium2 Bass kernel for nn_ConditionalDLFactorized17 (moe_routing).

Math (reference):
    logits = einsum('tbc,ec->tbe', x, assign_w) + assign_b      # router
    resp   = softmax(logits, -1)
    importance = resp.sum over tokens;  loss = .01*std(imp,ddof=1)/mean(imp)
    y = einsum('tbe,eoi,tbi->tbo', resp, pw_w1.reshape(e,o,i), x) + pw_B

Strategy: data-parallel over tokens (T*B = 16384 -> 2048/core on 8 cores),
pw_w1 replicated.  Per core, per 128-token tile:
    H_e = x_tile @ W_e^T  (f32r matmuls, K=512, accumulated in PSUM)
    y_tile += resp[:, e] * H_e   (DVE scalar_tensor_tensor, fused mul-add)
Host pre-transposes x and pw_w1 (partition-major layouts for wide DMA
packets) so no on-device transposes are needed for the main contraction.
Router logits are computed transposed (awT stationary: 16-column weight
load) and flipped back with DVE 32x32 stream transposes, keeping the PE
almost exclusively on the 1024 main matmuls.  The scalar importance
reduction is finished on host from per-core partials.
"""
import threading

import numpy as np

import concourse.bass as bass
import concourse.mybir as mybir
import concourse.tile as tile
from concourse import bacc
from concourse.bass_utils import run_bass_kernel_spmd

F32 = mybir.dt.float32
F32R = mybir.dt.float32r

T, B, C, OUT, NE = 2048, 8, 512, 512, 16
NCORES = 8
TOKENS = T * B                  # 16384
TOK_CORE = TOKENS // NCORES     # 2048
P = 128
NTILE = TOK_CORE // P           # 16
CB = C // P                     # 4 contraction blocks
NQUAD = TOK_CORE // 512         # router quads (512 tokens each)
LOSS_SCALE = 0.01

_lock = threading.Lock()
_cache = {}


def _build():
    from contextlib import ExitStack

    nc = bacc.Bacc()
    xT = nc.dram_tensor("xT", [NTILE, P, CB * P], F32R, kind="ExternalInput")
    w2 = nc.dram_tensor("w2", [NE, P, CB * OUT], F32R, kind="ExternalInput")
    awT = nc.dram_tensor("awT", [CB, P, NE], F32R, kind="ExternalInput")
    abT = nc.dram_tensor("abT", [NE, 1], F32, kind="ExternalInput")
    pwB_rep = nc.dram_tensor("pwB_rep", [P, OUT], F32, kind="ExternalInput")
    y = nc.dram_tensor("y", [TOK_CORE, OUT], F32, kind="ExternalOutput")
    imp = nc.dram_tensor("imp", [1, NE], F32, kind="ExternalOutput")

    with tile.TileContext(nc) as tc, ExitStack() as ctx:
        const = ctx.enter_context(tc.tile_pool(name="const", bufs=1))
        ypool = ctx.enter_context(tc.tile_pool(name="ypool", bufs=1))
        spool = ctx.enter_context(tc.tile_pool(name="spool", bufs=3))
        ps_h = ctx.enter_context(tc.tile_pool(name="ps_h", bufs=5, space="PSUM"))
        ps_l = ctx.enter_context(tc.tile_pool(name="ps_l", bufs=2, space="PSUM"))
        ps_i = ctx.enter_context(tc.tile_pool(name="ps_i", bufs=1, space="PSUM"))

        # ---- resident tensors; DMA issue order == priority order ----
        xt_sb = const.tile([P, NTILE, CB, P], F32R, tag="xt")
        awT_sb = const.tile([P, CB, NE], F32R, tag="awT")
        nc.sync.dma_start(awT_sb[:], awT.ap().rearrange("cb p e -> p cb e"))
        abT_sb = const.tile([NE, 1], F32, tag="abT")
        nc.sync.dma_start(abT_sb[:], abT.ap())
        pwB_sb = const.tile([P, OUT], F32, tag="pwB")
        nc.sync.dma_start(pwB_sb[:], pwB_rep.ap())
        w2_sb = const.tile([P, NE, CB, OUT], F32R, tag="w2")

        def load_xt(t):
            nc.sync.dma_start(
                xt_sb[:, t].rearrange("p cb k -> p (cb k)"), xT.ap()[t]
            )

        def load_w2(e):
            nc.sync.dma_start(
                w2_sb[:, e].rearrange("p cb o -> p (cb o)"), w2.ap()[e]
            )

        for t in range(4):
            load_xt(t)
        load_w2(0)
        for t in range(4, NTILE):
            load_xt(t)
        for e in range(1, NE):
            load_w2(e)
        ones_sb = const.tile([P, 1], F32, tag="ones")
        nc.vector.memset(ones_sb[:], 1.0)
        racc_sb = const.tile([P, NE], F32, tag="racc")
        nc.vector.memset(racc_sb[:], 0.0)

        # ---- router (transposed): lgT[e, tok] = awT.T @ xT, 512 tok/quad ----
        resps = [None] * NTILE
        yts = [None] * NTILE
        for q in range(NQUAD):
            lgT_ps = ps_l.tile([NE, 512], F32, tag="lgT")
            mov = xt_sb[:, 4 * q : 4 * q + 4]  # [128, 4, CB, 128]
            for cb in range(CB):
                nc.tensor.matmul(
                    lgT_ps[:], awT_sb[:, cb, :], mov[:, :, cb, :],
                    start=(cb == 0), stop=(cb == CB - 1),
                )
            # evacuate with router bias added (per-partition bias = per-expert)
            lgT = spool.tile([32, 512], F32, tag="lgT_sb")
            nc.vector.memset(lgT[:], 0.0)
            nc.scalar.activation(
                lgT[:NE, :], lgT_ps[:], mybir.ActivationFunctionType.Identity,
                bias=abT_sb[:], scale=1.0,
            )
            for dt_ in range(4):
                t = 4 * q + dt_
                # flip [16, 128] -> [128, 16] with four 32x32 DVE transposes
                lg = spool.tile([P, 32], F32, tag=f"lg{t % 2}")
                for j in range(4):
                    nc.vector.transpose(
                        lg[32 * j : 32 * j + 32, :],
                        lgT[:, 128 * dt_ + 32 * j : 128 * dt_ + 32 * j + 32],
                    )
                mx = spool.tile([P, 1], F32, tag="mx")
                nc.vector.reduce_max(
                    mx[:], lg[:, :NE], axis=mybir.AxisListType.X
                )
                negm = spool.tile([P, 1], F32, tag="negm")
                nc.vector.tensor_scalar_mul(negm[:], mx[:], -1.0)
                expt = spool.tile([P, NE], F32, tag="expt")
                ssum = spool.tile([P, 1], F32, tag="ssum")
                nc.scalar.activation(
                    expt[:], lg[:, :NE], mybir.ActivationFunctionType.Exp,
                    bias=negm[:], scale=1.0, accum_out=ssum[:],
                )
                rinv = spool.tile([P, 1], F32, tag="rinv")
                nc.vector.reciprocal(rinv[:], ssum[:])
                resp = spool.tile([P, NE], F32, tag=f"resp{t}")
                nc.vector.tensor_scalar_mul(resp[:], expt[:], rinv[:])
                nc.vector.tensor_add(racc_sb[:], racc_sb[:], resp[:])
                resps[t] = resp
                yt = ypool.tile([P, OUT], F32, tag=f"yt{t}")
                nc.vector.tensor_copy(yt[:], pwB_sb[:])
                yts[t] = yt

        # ---- importance partials (hides under the e-loop) ----
        imp_ps = ps_i.tile([1, NE], F32, tag="imp")
        nc.tensor.matmul(imp_ps[:], ones_sb[:], racc_sb[:], start=True, stop=True)
        imp_sb = spool.tile([1, NE], F32, tag="impsb")
        nc.vector.tensor_copy(imp_sb[:], imp_ps[:])
        nc.sync.dma_start(imp.ap(), imp_sb[:])

        # ---- main loop, expert-outer: consume w2 chunks as they land ----
        for e in range(NE):
            for t in range(NTILE):
                xt = xt_sb[:, t]
                h_ps = ps_h.tile([P, OUT], F32, tag="h")
                for cb in range(CB):
                    nc.tensor.matmul(
                        h_ps[:], xt[:, cb, :], w2_sb[:, e, cb, :],
                        start=(cb == 0), stop=(cb == CB - 1),
                    )
                nc.vector.scalar_tensor_tensor(
                    yts[t][:], h_ps[:], resps[t][:, e : e + 1], yts[t][:],
                    op0=mybir.AluOpType.mult, op1=mybir.AluOpType.add,
                )
                if e == NE - 1:
                    nc.sync.dma_start(
                        y.ap()[t * P : (t + 1) * P, :], yts[t][:]
                    )

    nc.compile()
    return nc


def _get_nc():
    with _lock:
        if "nc" not in _cache:
            _cache["nc"] = _build()
        return _cache["nc"]


def kernel(x, assign_w, assign_b, pw_w1, pw_B, _want_results=False):
    x = np.asarray(x, dtype=np.float32)
    assign_w = np.asarray(assign_w, dtype=np.float32)
    assign_b = np.asarray(assign_b, dtype=np.float32)
    pw_w1 = np.asarray(pw_w1, dtype=np.float32)
    pw_B = np.asarray(pw_B, dtype=np.float32)

    nc = _get_nc()

    # ---- host-side sharding / layout prep (partition-major for wide DMA) ----
    xt_full = x.reshape(TOKENS, C)
    # w2[e, p, cb*512 + o] = W'[e, i=cb*128+p, o] = pw_w1[e, o*512 + cb*128+p]
    w2_host = np.ascontiguousarray(
        pw_w1.reshape(NE, OUT, CB, P).transpose(0, 3, 2, 1)
    ).reshape(NE, P, CB * OUT)
    awT_host = np.ascontiguousarray(assign_w.T).reshape(CB, P, NE)
    abT_host = np.ascontiguousarray(assign_b.reshape(NE, 1))
    pwB_host = np.ascontiguousarray(np.tile(pw_B.reshape(1, OUT), (P, 1)))

    in_maps = []
    for k in range(NCORES):
        shard = xt_full[k * TOK_CORE : (k + 1) * TOK_CORE]          # [2048, 512]
        # xT[t, p, cb*128 + k] = x[t*128+k, cb*128+p]
        xT_host = np.ascontiguousarray(
            shard.reshape(NTILE, P, CB, P).transpose(0, 3, 2, 1)
        ).reshape(NTILE, P, CB * P)
        in_maps.append(
            {
                "xT": xT_host,
                "w2": w2_host,
                "awT": awT_host,
                "abT": abT_host,
                "pwB_rep": pwB_host,
            }
        )

    res = run_bass_kernel_spmd(nc, in_maps, core_ids=list(range(NCORES)))

    y = np.concatenate([r["y"] for r in res.results], axis=0).reshape(T, B, OUT)
    importance = np.sum(
        np.stack([r["imp"][0] for r in res.results]).astype(np.float64), axis=0
    )
    loss = np.float32(
        LOSS_SCALE * np.std(importance, ddof=1) / np.mean(importance)
    )
    if _want_results:
        return (y, loss), res
    return y, loss


# revision 12
# speedup vs baseline: 1.0132x; 1.0132x over previous
"""Trainium2 Bass kernel for nn_ConditionalDLFactorized17 (moe_routing).

Math (reference):
    logits = einsum('tbc,ec->tbe', x, assign_w) + assign_b      # router
    resp   = softmax(logits, -1)
    importance = resp.sum over tokens;  loss = .01*std(imp,ddof=1)/mean(imp)
    y = einsum('tbe,eoi,tbi->tbo', resp, pw_w1.reshape(e,o,i), x) + pw_B

Strategy: data-parallel over tokens (T*B = 16384 -> 2048/core on 8 cores),
pw_w1 replicated.  Per core, per 128-token tile:
    H_e = x_tile @ W_e^T  (f32r matmuls, K=512, accumulated in PSUM)
    y_tile += resp[:, e] * H_e   (DVE scalar_tensor_tensor, fused mul-add)
Host pre-transposes x and pw_w1 (partition-major layouts for wide DMA
packets) so no on-device transposes are needed for the main contraction.
Router logits are computed transposed (awT stationary: 16-column weight
load) and flipped back with DVE 32x32 stream transposes, keeping the PE
almost exclusively on the 1024 main matmuls.  The scalar importance
reduction is finished on host from per-core partials.
"""
import threading

import numpy as np

import concourse.bass as bass
import concourse.mybir as mybir
import concourse.tile as tile
from concourse import bacc
from concourse.bass_utils import run_bass_kernel_spmd

F32 = mybir.dt.float32
F32R = mybir.dt.float32r

T, B, C, OUT, NE = 2048, 8, 512, 512, 16
NCORES = 8
TOKENS = T * B                  # 16384
TOK_CORE = TOKENS // NCORES     # 2048
P = 128
NTILE = TOK_CORE // P           # 16
CB = C // P                     # 4 contraction blocks
NQUAD = TOK_CORE // 512         # router quads (512 tokens each)
LOSS_SCALE = 0.01

_lock = threading.Lock()
_cache = {}


def _build():
    from contextlib import ExitStack

    nc = bacc.Bacc()
    xT = nc.dram_tensor("xT", [NQUAD, P, 4 * CB * P], F32R, kind="ExternalInput")
    w2 = nc.dram_tensor("w2", [NE, P, CB * OUT], F32R, kind="ExternalInput")
    awT = nc.dram_tensor("awT", [CB, P, NE], F32R, kind="ExternalInput")
    abT = nc.dram_tensor("abT", [NE, 1], F32, kind="ExternalInput")
    pwB_rep = nc.dram_tensor("pwB_rep", [P, OUT], F32, kind="ExternalInput")
    y = nc.dram_tensor("y", [TOK_CORE, OUT], F32, kind="ExternalOutput")
    imp = nc.dram_tensor("imp", [1, NE], F32, kind="ExternalOutput")

    with tile.TileContext(nc) as tc, ExitStack() as ctx:
        const = ctx.enter_context(tc.tile_pool(name="const", bufs=1))
        ypool = ctx.enter_context(tc.tile_pool(name="ypool", bufs=1))
        spool = ctx.enter_context(tc.tile_pool(name="spool", bufs=3))
        ps_h = ctx.enter_context(tc.tile_pool(name="ps_h", bufs=6, space="PSUM"))
        ps_l = ctx.enter_context(tc.tile_pool(name="ps_l", bufs=1, space="PSUM"))
        ps_i = ctx.enter_context(tc.tile_pool(name="ps_i", bufs=1, space="PSUM"))

        # ---- resident tensors; DMA issue order == priority order ----
        xt_sb = const.tile([P, NTILE, CB, P], F32R, tag="xt")
        awT_sb = const.tile([P, CB, NE], F32R, tag="awT")
        nc.sync.dma_start(awT_sb[:], awT.ap().rearrange("cb p e -> p cb e"))
        abT_sb = const.tile([NE, 1], F32, tag="abT")
        nc.sync.dma_start(abT_sb[:], abT.ap())
        pwB_sb = const.tile([P, OUT], F32, tag="pwB")
        nc.sync.dma_start(pwB_sb[:], pwB_rep.ap())
        w2_sb = const.tile([P, NE, CB, OUT], F32R, tag="w2")

        def load_xt(q):
            nc.sync.dma_start(
                xt_sb[:, 4 * q : 4 * q + 4].rearrange("p t cb k -> p (t cb k)"),
                xT.ap()[q],
            )

        def load_w2(e):
            nc.sync.dma_start(
                w2_sb[:, e].rearrange("p cb o -> p (cb o)"), w2.ap()[e]
            )

        ones_sb = const.tile([P, 1], F32, tag="ones")
        nc.vector.memset(ones_sb[:], 1.0)
        load_xt(0)
        load_w2(0)
        for q in range(1, NQUAD):
            load_xt(q)
        for e in range(1, NE):
            load_w2(e)

        # ---- router (transposed): lgT[e, tok] = awT.T @ xT, 512 tok/quad ----
        imp_ps = ps_i.tile([1, NE], F32, tag="imp")
        resps = [None] * NTILE
        yts = [None] * NTILE
        for q in range(NQUAD):
            lgT_ps = ps_l.tile([NE, 512], F32, tag="lgT")
            mov = xt_sb[:, 4 * q : 4 * q + 4]  # [128, 4, CB, 128]
            for cb in range(CB):
                nc.tensor.matmul(
                    lgT_ps[:], awT_sb[:, cb, :], mov[:, :, cb, :],
                    start=(cb == 0), stop=(cb == CB - 1),
                )
            # evacuate with router bias added (per-partition bias = per-expert)
            lgT = spool.tile([32, 512], F32, tag="lgT_sb")
            nc.vector.memset(lgT[:], 0.0)
            nc.scalar.activation(
                lgT[:NE, :], lgT_ps[:], mybir.ActivationFunctionType.Identity,
                bias=abT_sb[:], scale=1.0,
            )
            for dt_ in range(4):
                t = 4 * q + dt_
                # flip [16, 128] -> [128, 16] with four 32x32 DVE transposes
                lg = spool.tile([P, 32], F32, tag=f"lg{t % 2}")
                for j in range(4):
                    nc.vector.transpose(
                        lg[32 * j : 32 * j + 32, :],
                        lgT[:, 128 * dt_ + 32 * j : 128 * dt_ + 32 * j + 32],
                    )
                mx = spool.tile([P, 1], F32, tag="mx")
                nc.vector.reduce_max(
                    mx[:], lg[:, :NE], axis=mybir.AxisListType.X
                )
                negm = spool.tile([P, 1], F32, tag="negm")
                nc.scalar.mul(negm[:], mx[:], -1.0)
                expt = spool.tile([P, NE], F32, tag="expt")
                ssum = spool.tile([P, 1], F32, tag="ssum")
                nc.scalar.activation(
                    expt[:], lg[:, :NE], mybir.ActivationFunctionType.Exp,
                    bias=negm[:], scale=1.0, accum_out=ssum[:],
                )
                rinv = spool.tile([P, 1], F32, tag="rinv")
                nc.vector.reciprocal(rinv[:], ssum[:])
                resp = spool.tile([P, NE], F32, tag=f"resp{t}")
                nc.vector.tensor_scalar_mul(resp[:], expt[:], rinv[:])
                nc.tensor.matmul(
                    imp_ps[:], ones_sb[:], resp[:],
                    start=(t == 0), stop=(t == NTILE - 1),
                )
                resps[t] = resp
                yt = ypool.tile([P, OUT], F32, tag=f"yt{t}")
                nc.scalar.copy(yt[:], pwB_sb[:])
                yts[t] = yt

        # ---- importance partials (hides under the e-loop) ----
        imp_sb = spool.tile([1, NE], F32, tag="impsb")
        nc.vector.tensor_copy(imp_sb[:], imp_ps[:])
        nc.sync.dma_start(imp.ap(), imp_sb[:])

        # ---- main loop, expert-outer: consume w2 chunks as they land ----
        for e in range(NE):
            for t in range(NTILE):
                xt = xt_sb[:, t]
                h_ps = ps_h.tile([P, OUT], F32, tag="h")
                for cb in range(CB):
                    nc.tensor.matmul(
                        h_ps[:], xt[:, cb, :], w2_sb[:, e, cb, :],
                        start=(cb == 0), stop=(cb == CB - 1),
                    )
                nc.vector.scalar_tensor_tensor(
                    yts[t][:], h_ps[:], resps[t][:, e : e + 1], yts[t][:],
                    op0=mybir.AluOpType.mult, op1=mybir.AluOpType.add,
                )
                if e == NE - 1:
                    nc.sync.dma_start(
                        y.ap()[t * P : (t + 1) * P, :], yts[t][:]
                    )

    nc.compile()
    return nc


def _get_nc():
    with _lock:
        if "nc" not in _cache:
            _cache["nc"] = _build()
        return _cache["nc"]


def kernel(x, assign_w, assign_b, pw_w1, pw_B, _want_results=False):
    x = np.asarray(x, dtype=np.float32)
    assign_w = np.asarray(assign_w, dtype=np.float32)
    assign_b = np.asarray(assign_b, dtype=np.float32)
    pw_w1 = np.asarray(pw_w1, dtype=np.float32)
    pw_B = np.asarray(pw_B, dtype=np.float32)

    nc = _get_nc()

    # ---- host-side sharding / layout prep (partition-major for wide DMA) ----
    xt_full = x.reshape(TOKENS, C)
    # w2[e, p, cb*512 + o] = W'[e, i=cb*128+p, o] = pw_w1[e, o*512 + cb*128+p]
    w2_host = np.ascontiguousarray(
        pw_w1.reshape(NE, OUT, CB, P).transpose(0, 3, 2, 1)
    ).reshape(NE, P, CB * OUT)
    awT_host = np.ascontiguousarray(assign_w.T).reshape(CB, P, NE)
    abT_host = np.ascontiguousarray(assign_b.reshape(NE, 1))
    pwB_host = np.ascontiguousarray(np.tile(pw_B.reshape(1, OUT), (P, 1)))

    in_maps = []
    for k in range(NCORES):
        shard = xt_full[k * TOK_CORE : (k + 1) * TOK_CORE]          # [2048, 512]
        # xT[t, p, cb*128 + k] = x[t*128+k, cb*128+p]
        xT_host = np.ascontiguousarray(
            shard.reshape(NQUAD, 4, P, CB, P).transpose(0, 4, 1, 3, 2)
        ).reshape(NQUAD, P, 4 * CB * P)
        in_maps.append(
            {
                "xT": xT_host,
                "w2": w2_host,
                "awT": awT_host,
                "abT": abT_host,
                "pwB_rep": pwB_host,
            }
        )

    res = run_bass_kernel_spmd(nc, in_maps, core_ids=list(range(NCORES)))

    y = np.concatenate([r["y"] for r in res.results], axis=0).reshape(T, B, OUT)
    importance = np.sum(
        np.stack([r["imp"][0] for r in res.results]).astype(np.float64), axis=0
    )
    loss = np.float32(
        LOSS_SCALE * np.std(importance, ddof=1) / np.mean(importance)
    )
    if _want_results:
        return (y, loss), res
    return y, loss


# revision 13
# speedup vs baseline: 1.0291x; 1.0157x over previous
"""Trainium2 Bass kernel for nn_ConditionalDLFactorized17 (moe_routing).

Math (reference):
    logits = einsum('tbc,ec->tbe', x, assign_w) + assign_b      # router
    resp   = softmax(logits, -1)
    importance = resp.sum over tokens;  loss = .01*std(imp,ddof=1)/mean(imp)
    y = einsum('tbe,eoi,tbi->tbo', resp, pw_w1.reshape(e,o,i), x) + pw_B

Strategy: data-parallel over tokens (T*B = 16384 -> 2048/core on 8 cores),
pw_w1 replicated.  Per core, per 128-token tile:
    H_e = x_tile @ W_e^T  (f32r matmuls, K=512, accumulated in PSUM)
    y_tile += resp[:, e] * H_e   (DVE scalar_tensor_tensor, fused mul-add)
Host pre-transposes x and pw_w1 (partition-major layouts for wide DMA
packets) so no on-device transposes are needed for the main contraction.
Router logits are computed transposed (awT stationary: 16-column weight
load) and flipped back with DVE 32x32 stream transposes, keeping the PE
almost exclusively on the 1024 main matmuls.  The scalar importance
reduction is finished on host from per-core partials.
"""
import threading

import ml_dtypes
import numpy as np

import concourse.bass as bass
import concourse.mybir as mybir
import concourse.tile as tile
from concourse import bacc
from concourse.bass_utils import run_bass_kernel_spmd

F32 = mybir.dt.float32
F32R = mybir.dt.float32r
BF16 = mybir.dt.bfloat16

T, B, C, OUT, NE = 2048, 8, 512, 512, 16
NCORES = 8
TOKENS = T * B                  # 16384
TOK_CORE = TOKENS // NCORES     # 2048
P = 128
NTILE = TOK_CORE // P           # 16
CB = C // P                     # 4 contraction blocks
NQUAD = TOK_CORE // 512         # router quads (512 tokens each)
LOSS_SCALE = 0.01

_lock = threading.Lock()
_cache = {}


def _build():
    from contextlib import ExitStack

    nc = bacc.Bacc()
    xT = nc.dram_tensor("xT", [NQUAD, P, 4 * CB * P], BF16, kind="ExternalInput")
    xTr = nc.dram_tensor("xTr", [NQUAD, P, 4 * CB * P], F32R, kind="ExternalInput")
    w2 = nc.dram_tensor("w2", [NE, P, CB * OUT], BF16, kind="ExternalInput")
    awT = nc.dram_tensor("awT", [CB, P, NE], F32R, kind="ExternalInput")
    abT = nc.dram_tensor("abT", [NE, 1], F32, kind="ExternalInput")
    pwB_rep = nc.dram_tensor("pwB_rep", [P, OUT], F32, kind="ExternalInput")
    y = nc.dram_tensor("y", [TOK_CORE, OUT], F32, kind="ExternalOutput")
    imp = nc.dram_tensor("imp", [1, NE], F32, kind="ExternalOutput")

    with tile.TileContext(nc) as tc, ExitStack() as ctx:
        const = ctx.enter_context(tc.tile_pool(name="const", bufs=1))
        ypool = ctx.enter_context(tc.tile_pool(name="ypool", bufs=1))
        spool = ctx.enter_context(tc.tile_pool(name="spool", bufs=3))
        ps_h = ctx.enter_context(tc.tile_pool(name="ps_h", bufs=6, space="PSUM"))
        ps_l = ctx.enter_context(tc.tile_pool(name="ps_l", bufs=1, space="PSUM"))
        ps_i = ctx.enter_context(tc.tile_pool(name="ps_i", bufs=1, space="PSUM"))

        # ---- resident tensors; DMA issue order == priority order ----
        xt_sb = const.tile([P, NTILE, CB, P], BF16, tag="xt")
        xtr_sb = const.tile([P, NTILE, CB, P], F32R, tag="xtr")
        awT_sb = const.tile([P, CB, NE], F32R, tag="awT")
        nc.sync.dma_start(awT_sb[:], awT.ap().rearrange("cb p e -> p cb e"))
        abT_sb = const.tile([NE, 1], F32, tag="abT")
        nc.sync.dma_start(abT_sb[:], abT.ap())
        pwB_sb = const.tile([P, OUT], F32, tag="pwB")
        nc.sync.dma_start(pwB_sb[:], pwB_rep.ap())
        w2_sb = const.tile([P, NE, CB, OUT], BF16, tag="w2")

        def load_xt(q):
            nc.sync.dma_start(
                xt_sb[:, 4 * q : 4 * q + 4].rearrange("p t cb k -> p (t cb k)"),
                xT.ap()[q],
            )
            nc.sync.dma_start(
                xtr_sb[:, 4 * q : 4 * q + 4].rearrange("p t cb k -> p (t cb k)"),
                xTr.ap()[q],
            )

        def load_w2(e):
            nc.sync.dma_start(
                w2_sb[:, e].rearrange("p cb o -> p (cb o)"), w2.ap()[e]
            )

        ones_sb = const.tile([P, 1], F32, tag="ones")
        nc.vector.memset(ones_sb[:], 1.0)
        load_xt(0)
        load_w2(0)
        for q in range(1, NQUAD):
            load_xt(q)
        for e in range(1, NE):
            load_w2(e)

        # ---- router (transposed): lgT[e, tok] = awT.T @ xT, 512 tok/quad ----
        imp_ps = ps_i.tile([1, NE], F32, tag="imp")
        resps = [None] * NTILE
        yts = [None] * NTILE
        for q in range(NQUAD):
            lgT_ps = ps_l.tile([NE, 512], F32, tag="lgT")
            mov = xtr_sb[:, 4 * q : 4 * q + 4]  # [128, 4, CB, 128]
            for cb in range(CB):
                nc.tensor.matmul(
                    lgT_ps[:], awT_sb[:, cb, :], mov[:, :, cb, :],
                    start=(cb == 0), stop=(cb == CB - 1),
                )
            # evacuate with router bias added (per-partition bias = per-expert)
            lgT = spool.tile([32, 512], F32, tag="lgT_sb")
            nc.vector.memset(lgT[:], 0.0)
            nc.scalar.activation(
                lgT[:NE, :], lgT_ps[:], mybir.ActivationFunctionType.Identity,
                bias=abT_sb[:], scale=1.0,
            )
            for dt_ in range(4):
                t = 4 * q + dt_
                # flip [16, 128] -> [128, 16] with four 32x32 DVE transposes
                lg = spool.tile([P, 32], F32, tag=f"lg{t % 2}")
                for j in range(4):
                    nc.vector.transpose(
                        lg[32 * j : 32 * j + 32, :],
                        lgT[:, 128 * dt_ + 32 * j : 128 * dt_ + 32 * j + 32],
                    )
                mx = spool.tile([P, 1], F32, tag="mx")
                nc.vector.reduce_max(
                    mx[:], lg[:, :NE], axis=mybir.AxisListType.X
                )
                negm = spool.tile([P, 1], F32, tag="negm")
                nc.scalar.mul(negm[:], mx[:], -1.0)
                expt = spool.tile([P, NE], F32, tag="expt")
                ssum = spool.tile([P, 1], F32, tag="ssum")
                nc.scalar.activation(
                    expt[:], lg[:, :NE], mybir.ActivationFunctionType.Exp,
                    bias=negm[:], scale=1.0, accum_out=ssum[:],
                )
                rinv = spool.tile([P, 1], F32, tag="rinv")
                nc.vector.reciprocal(rinv[:], ssum[:])
                resp = spool.tile([P, NE], F32, tag=f"resp{t}")
                nc.vector.tensor_scalar_mul(resp[:], expt[:], rinv[:])
                nc.tensor.matmul(
                    imp_ps[:], ones_sb[:], resp[:],
                    start=(t == 0), stop=(t == NTILE - 1),
                )
                resps[t] = resp
                yt = ypool.tile([P, OUT], F32, tag=f"yt{t}")
                nc.scalar.copy(yt[:], pwB_sb[:])
                yts[t] = yt

        # ---- importance partials (hides under the e-loop) ----
        imp_sb = spool.tile([1, NE], F32, tag="impsb")
        nc.vector.tensor_copy(imp_sb[:], imp_ps[:])
        nc.sync.dma_start(imp.ap(), imp_sb[:])

        # ---- main loop, expert-outer: consume w2 chunks as they land ----
        for e in range(NE):
            for t in range(NTILE):
                xt = xt_sb[:, t]
                h_ps = ps_h.tile([P, OUT], F32, tag="h")
                for cb in range(CB):
                    nc.tensor.matmul(
                        h_ps[:], xt[:, cb, :], w2_sb[:, e, cb, :],
                        start=(cb == 0), stop=(cb == CB - 1),
                    )
                scr = spool.tile([P, OUT], F32, tag=f"scr{e % 3}")
                nc.scalar.activation(
                    scr[:], h_ps[:], mybir.ActivationFunctionType.Copy,
                    scale=resps[t][:, e : e + 1],
                )
                eng = nc.gpsimd if (e % 4 == 3) else nc.vector
                eng.tensor_add(yts[t][:], yts[t][:], scr[:])
                if e == NE - 1:
                    nc.sync.dma_start(
                        y.ap()[t * P : (t + 1) * P, :], yts[t][:]
                    )

    nc.compile()
    return nc


def _get_nc():
    with _lock:
        if "nc" not in _cache:
            _cache["nc"] = _build()
        return _cache["nc"]


def kernel(x, assign_w, assign_b, pw_w1, pw_B, _want_results=False):
    x = np.asarray(x, dtype=np.float32)
    assign_w = np.asarray(assign_w, dtype=np.float32)
    assign_b = np.asarray(assign_b, dtype=np.float32)
    pw_w1 = np.asarray(pw_w1, dtype=np.float32)
    pw_B = np.asarray(pw_B, dtype=np.float32)

    nc = _get_nc()

    # ---- host-side sharding / layout prep (partition-major for wide DMA) ----
    xt_full = x.reshape(TOKENS, C)
    # w2[e, p, cb*512 + o] = W'[e, i=cb*128+p, o] = pw_w1[e, o*512 + cb*128+p]
    w2_host = np.ascontiguousarray(
        pw_w1.reshape(NE, OUT, CB, P).transpose(0, 3, 2, 1)
    ).reshape(NE, P, CB * OUT).astype(ml_dtypes.bfloat16)
    awT_host = np.ascontiguousarray(assign_w.T).reshape(CB, P, NE)
    abT_host = np.ascontiguousarray(assign_b.reshape(NE, 1))
    pwB_host = np.ascontiguousarray(np.tile(pw_B.reshape(1, OUT), (P, 1)))

    in_maps = []
    for k in range(NCORES):
        shard = xt_full[k * TOK_CORE : (k + 1) * TOK_CORE]          # [2048, 512]
        # xT[t, p, cb*128 + k] = x[t*128+k, cb*128+p]
        xT_host = np.ascontiguousarray(
            shard.reshape(NQUAD, 4, P, CB, P).transpose(0, 4, 1, 3, 2)
        ).reshape(NQUAD, P, 4 * CB * P)
        in_maps.append(
            {
                "xT": xT_host.astype(ml_dtypes.bfloat16),
                "xTr": xT_host,
                "w2": w2_host,
                "awT": awT_host,
                "abT": abT_host,
                "pwB_rep": pwB_host,
            }
        )

    res = run_bass_kernel_spmd(nc, in_maps, core_ids=list(range(NCORES)))

    y = np.concatenate([r["y"] for r in res.results], axis=0).reshape(T, B, OUT)
    importance = np.sum(
        np.stack([r["imp"][0] for r in res.results]).astype(np.float64), axis=0
    )
    loss = np.float32(
        LOSS_SCALE * np.std(importance, ddof=1) / np.mean(importance)
    )
    if _want_results:
        return (y, loss), res
    return y, loss
